# revision 1
# baseline (speedup 1.0000x reference)
"""
Trainium2 Bass kernel for nn_MF_MGCN (5-band 2-layer GCN + MLP head).

Strategy (data-parallel over graphs, 8 NeuronCores):
  * Every graph has 19 nodes; edges never cross graphs.  Graph-local
    aggregation is done as dense block-diagonal matmuls with 6 graphs
    (114 rows) per block on the TensorEngine.
  * GCN layer 1 has a 1-channel input per band, so its [N,32] hidden is
    rank-1; only the scalar aggregate s = A_f_norm @ x is needed per band.
    With bt1 == 0, relu(BN1) @ W2 collapses exactly onto the two features
    (relu(z), relu(-z)), z = s - mean(s).  GCN layer 2 then aggregates
    just 2 channels per band.
  * BatchNorm uses global batch statistics, so the pipeline runs as four
    device launches with tiny partial-sum tensors combined on the host
    between launches (host does scalar/statistics math + layout only).
  * The structural edge pattern is identical for every graph (reference
    generator uses one random pattern + offsets), so layer-2 aggregation
    uses one shared block-diagonal stationary matrix and wide moving
    operands (few large matmuls).  If that (or bt1==0) ever fails to
    hold, a pure-numpy fallback reproduces the reference exactly.
"""

import sys

sys.path.insert(0, "/opt/trn_rl_repo")

import numpy as np
import ml_dtypes

BF16 = ml_dtypes.bfloat16

# Problem constants (hardcoded per task contract).
B = 32768
NN = 19
N = B * NN
BANDS = 5
EF, ES = 120, 60
EPS = 1e-5
NCORES = 8
G = B // NCORES           # graphs per core = 4096
SLOT = 6                  # graphs per 114-row block
NBLK = (G + SLOT - 1) // SLOT   # 683 blocks per core
NSLOT = NBLK * SLOT       # 4098 slots (2 zero-pad graphs)
P114 = SLOT * NN          # 114
CH1 = 64                  # L1 blocks per psum chunk (320 fp32 cols)
NCH1 = (NBLK + CH1 - 1) // CH1
CH2 = 51                  # L2 blocks per matmul     (510 fp32 cols)

_KERNEL_CACHE = {}


# --------------------------------------------------------------------------
# numpy fallback (exact reference math) -- used only if structural
# assumptions are violated; keeps kernel() correct for any inputs.
# --------------------------------------------------------------------------
def _bn_np(h, g, b):
    m = h.mean(0)
    v = h.var(0)
    return (h - m) / np.sqrt(v + EPS) * g + b


def _gcn_np(h, W, b, src, dst, ew, n):
    h = h @ W
    deg = np.zeros(n, np.float64)
    np.add.at(deg, dst, ew)
    deg += 1.0
    dinv = 1.0 / np.sqrt(deg)
    norm = dinv[src] * ew * dinv[dst]
    agg = np.zeros_like(h, dtype=np.float64)
    np.add.at(agg, dst, norm[:, None] * h[src])
    return agg + (dinv * dinv)[:, None] * h + b


def _fallback_numpy(i):
    x = np.asarray(i["x"], np.float64)
    sf, df = np.asarray(i["edge_index_func"][0]), np.asarray(i["edge_index_func"][1])
    ss, ds = np.asarray(i["edge_index_struct"][0]), np.asarray(i["edge_index_struct"][1])
    ew = np.asarray(i["edge_weight_func"], np.float64)
    ews = np.ones(ss.shape[0], np.float64)
    n = x.shape[0]
    outs = []
    for b in range(BANDS):
        h = _gcn_np(x[:, b : b + 1], np.asarray(i["W1"][b], np.float64),
                    np.asarray(i["b1"][b], np.float64), sf, df, ew, n)
        h = np.maximum(_bn_np(h, np.asarray(i["g1"][b], np.float64),
                              np.asarray(i["bt1"][b], np.float64)), 0)
        h = _gcn_np(h, np.asarray(i["W2"][b], np.float64),
                    np.asarray(i["b2"][b], np.float64), ss, ds, ews, n)
        h = np.maximum(_bn_np(h, np.asarray(i["g2"][b], np.float64),
                              np.asarray(i["bt2"][b], np.float64)), 0)
        outs.append(h.reshape(n // NN, NN * 2))
    xc = np.concatenate(outs, axis=1)
    h = np.maximum(_bn_np(xc @ np.asarray(i["lin1_W"], np.float64)
                          + np.asarray(i["lin1_b"], np.float64),
                          np.asarray(i["g3"], np.float64),
                          np.asarray(i["bt3"], np.float64)), 0)
    h = np.maximum(h @ np.asarray(i["lin2_W"], np.float64)
                   + np.asarray(i["lin2_b"], np.float64), 0)
    out = h @ np.asarray(i["lin3_W"], np.float64) + np.asarray(i["lin3_b"], np.float64)
    return out.astype(np.float32)


# --------------------------------------------------------------------------
# Bass kernel builders
# --------------------------------------------------------------------------
def _get_bass():
    import concourse.bacc as bacc
    import concourse.mybir as mybir
    from concourse import tile
    return bacc, mybir, tile


def _build_l1(hb):
    """Func aggregation, single launch: af streamed in double-buffered chunks."""
    bass, mybir, tile = _get_bass()
    f32, bf16 = mybir.dt.float32, mybir.dt.bfloat16
    nc = bass.Bacc(None, target_bir_lowering=False)
    xb = nc.dram_tensor("xb", [P114, hb, BANDS], bf16, kind="ExternalInput")
    af = nc.dram_tensor("af", [P114, hb, 128], bf16, kind="ExternalInput")
    s_out = nc.dram_tensor("s_out", [128, hb, BANDS], f32, kind="ExternalOutput")
    st1 = nc.dram_tensor("st1", [16, 1], f32, kind="ExternalOutput")
    with tile.TileContext(nc) as tc:
        with (
            tc.tile_pool(name="const", bufs=1) as cp,
            tc.tile_pool(name="wt", bufs=3) as wp,
            tc.tile_pool(name="ps", bufs=4, space="PSUM") as pp,
            tc.tile_pool(name="big", bufs=1) as bp,
        ):
            x_t = cp.tile([P114, hb, BANDS], bf16)
            nc.sync.dma_start(x_t[:], xb[:])
            s_t = bp.tile([128, hb, BANDS], f32)
            nch = (hb + CH1 - 1) // CH1
            for c in range(nch):
                nb = min(CH1, hb - c * CH1)
                a_t = wp.tile([P114, CH1, 128], bf16, tag="af")
                nc.sync.dma_start(a_t[:, :nb, :], af[:, c * CH1 : c * CH1 + nb, :])
                ps = pp.tile([128, CH1, BANDS], f32, tag="ps")
                for j in range(nb):
                    nc.tensor.matmul(
                        ps[:, j, :],
                        a_t[:, j, :],
                        x_t[:, c * CH1 + j, :],
                        start=True,
                        stop=True,
                    )
                nc.vector.tensor_copy(
                    out=s_t[:, c * CH1 : c * CH1 + nb, :], in_=ps[:, :nb, :]
                )
            sq_t = bp.tile([128, hb, BANDS], f32)
            nc.vector.tensor_tensor(out=sq_t[:], in0=s_t[:], in1=s_t[:],
                                    op=mybir.AluOpType.mult)
            part = cp.tile([128, 16], f32)
            nc.vector.memset(part[:], 0.0)
            for b in range(BANDS):
                nc.vector.tensor_reduce(out=part[:, b : b + 1], in_=s_t[:, :, b],
                                        axis=mybir.AxisListType.X,
                                        op=mybir.AluOpType.add)
                nc.vector.tensor_reduce(out=part[:, 5 + b : 6 + b], in_=sq_t[:, :, b],
                                        axis=mybir.AxisListType.X,
                                        op=mybir.AluOpType.add)
            ones = cp.tile([128, 1], f32)
            nc.vector.memset(ones[:], 1.0)
            pst = pp.tile([16, 1], f32, tag="st")
            nc.tensor.matmul(pst[:], part[:], ones[:], start=True, stop=True)
            st1_t = cp.tile([16, 1], f32)
            nc.vector.tensor_copy(out=st1_t[:], in_=pst[:])
            nc.sync.dma_start(st1[:], st1_t[:])
            nc.sync.dma_start(s_out[:], s_t[:])
    nc.compile()
    return nc


def _build_l2():
    """u=(relu(z),relu(-z)); v = As_blockdiag_shared @ u; stats(v)."""
    bass, mybir, tile = _get_bass()
    f32, bf16 = mybir.dt.float32, mybir.dt.bfloat16
    nc = bass.Bacc(None, target_bir_lowering=False)
    s_in = nc.dram_tensor("s_in", [128, NBLK, BANDS], f32, kind="ExternalInput")
    mub = nc.dram_tensor("mub", [128, 1, BANDS], f32, kind="ExternalInput")
    asb = nc.dram_tensor("asb", [P114, 128], bf16, kind="ExternalInput")
    v_out = nc.dram_tensor("v_out", [128, NBLK, 10], f32, kind="ExternalOutput")
    st2 = nc.dram_tensor("st2", [32, 1], f32, kind="ExternalOutput")
    nch = (NBLK + CH2 - 1) // CH2
    with tile.TileContext(nc) as tc:
        with (
            tc.tile_pool(name="const", bufs=1) as cp,
            tc.tile_pool(name="ps", bufs=4, space="PSUM") as pp,
            tc.tile_pool(name="big", bufs=1) as bp,
        ):
            s_t = bp.tile([128, NBLK, BANDS], f32)
            nc.sync.dma_start(s_t[:], s_in[:])
            mu_t = cp.tile([128, 1, BANDS], f32)
            nc.sync.dma_start(mu_t[:], mub[:])
            as_t = cp.tile([P114, 128], bf16)
            nc.sync.dma_start(as_t[:], asb[:])
            # z = s - mu  (mu broadcast over blocks)
            z_t = bp.tile([128, NBLK, BANDS], f32)
            nc.vector.tensor_tensor(
                out=z_t[:], in0=s_t[:],
                in1=mu_t[:].to_broadcast([128, NBLK, BANDS]),
                op=mybir.AluOpType.subtract,
            )
            # u columns: [u+ bands 0..4 | u- bands 0..4]
            u_t = bp.tile([P114, NBLK, 10], bf16)
            nc.scalar.activation(u_t[:, :, 0:5], z_t[:P114],
                                 mybir.ActivationFunctionType.Relu)
            nc.scalar.activation(u_t[:, :, 5:10], z_t[:P114],
                                 mybir.ActivationFunctionType.Relu, scale=-1.0)
            v_t = bp.tile([128, NBLK, 10], f32)
            for c in range(nch):
                nb = min(CH2, NBLK - c * CH2)
                ps = pp.tile([128, CH2, 10], f32, tag="ps")
                nc.tensor.matmul(
                    ps[:, :nb, :],
                    as_t[:],
                    u_t[:, c * CH2 : c * CH2 + nb, :],
                    start=True,
                    stop=True,
                )
                nc.vector.tensor_copy(
                    out=v_t[:, c * CH2 : c * CH2 + nb, :], in_=ps[:, :nb, :]
                )
            # stats: for each band: sum v+, v-, v+^2, v-^2, v+*v-
            sq_t = bp.tile([128, NBLK, 10], f32)
            nc.vector.tensor_tensor(out=sq_t[:], in0=v_t[:], in1=v_t[:],
                                    op=mybir.AluOpType.mult)
            p01_t = bp.tile([128, NBLK, BANDS], f32)
            nc.vector.tensor_tensor(out=p01_t[:], in0=v_t[:, :, 0:5],
                                    in1=v_t[:, :, 5:10], op=mybir.AluOpType.mult)
            part = cp.tile([128, 32], f32)
            nc.vector.memset(part[:], 0.0)
            for b in range(BANDS):
                for k, src in (
                    (0, v_t[:, :, b]), (5, v_t[:, :, 5 + b]),
                    (10, sq_t[:, :, b]), (15, sq_t[:, :, 5 + b]),
                    (20, p01_t[:, :, b]),
                ):
                    nc.vector.tensor_reduce(out=part[:, k + b : k + b + 1], in_=src,
                                            axis=mybir.AxisListType.X,
                                            op=mybir.AluOpType.add)
            ones = cp.tile([128, 1], f32)
            nc.vector.memset(ones[:], 1.0)
            pst = pp.tile([32, 1], f32, tag="st")
            nc.tensor.matmul(pst[:], part[:], ones[:], start=True, stop=True)
            st2_t = cp.tile([32, 1], f32)
            nc.vector.tensor_copy(out=st2_t[:], in_=pst[:])
            nc.sync.dma_start(st2[:], st2_t[:])
            nc.sync.dma_start(v_out[:], v_t[:])
    nc.compile()
    return nc


def _build_l3():
    """xc_k = relu(A_k*v0 + B_k*v1 + C_k); y1 = lin1(xc); stats(y1)."""
    bass, mybir, tile = _get_bass()
    f32 = mybir.dt.float32
    nc = bass.Bacc(None, target_bir_lowering=False)
    v0p = nc.dram_tensor("v0p", [95, G], f32, kind="ExternalInput")
    v1p = nc.dram_tensor("v1p", [95, G], f32, kind="ExternalInput")
    coef = nc.dram_tensor("coef", [95, 8], f32, kind="ExternalInput")
    w1k0 = nc.dram_tensor("w1k0", [95, 128], f32, kind="ExternalInput")
    w1k1 = nc.dram_tensor("w1k1", [95, 128], f32, kind="ExternalInput")
    l1b = nc.dram_tensor("l1b", [128, 1], f32, kind="ExternalInput")
    y1 = nc.dram_tensor("y1", [128, G], f32, kind="ExternalOutput")
    st3 = nc.dram_tensor("st3", [128, 2], f32, kind="ExternalOutput")
    with tile.TileContext(nc) as tc:
        with (
            tc.tile_pool(name="const", bufs=1) as cp,
            tc.tile_pool(name="ps", bufs=4, space="PSUM") as pp,
            tc.tile_pool(name="big", bufs=1) as bp,
        ):
            v0_t = bp.tile([95, G], f32)
            v1_t = bp.tile([95, G], f32)
            nc.sync.dma_start(v0_t[:], v0p[:])
            nc.sync.dma_start(v1_t[:], v1p[:])
            co_t = cp.tile([95, 8], f32)
            nc.sync.dma_start(co_t[:], coef[:])
            w0_t = cp.tile([95, 128], f32)
            w1_t = cp.tile([95, 128], f32)
            nc.sync.dma_start(w0_t[:], w1k0[:])
            nc.sync.dma_start(w1_t[:], w1k1[:])
            b_t = cp.tile([128, 1], f32)
            nc.sync.dma_start(b_t[:], l1b[:])
            xc = []
            for k in range(2):
                t0 = bp.tile([95, G], f32, tag=f"t0{k}")
                nc.vector.tensor_scalar(out=t0[:], in0=v0_t[:],
                                        scalar1=co_t[:, 3 * k : 3 * k + 1],
                                        scalar2=None, op0=mybir.AluOpType.mult)
                t1 = bp.tile([95, G], f32, tag=f"t1{k}")
                nc.vector.tensor_scalar(out=t1[:], in0=v1_t[:],
                                        scalar1=co_t[:, 3 * k + 1 : 3 * k + 2],
                                        scalar2=None, op0=mybir.AluOpType.mult)
                nc.vector.tensor_tensor(out=t0[:], in0=t0[:], in1=t1[:],
                                        op=mybir.AluOpType.add)
                nc.scalar.activation(t0[:], t0[:],
                                     mybir.ActivationFunctionType.Relu,
                                     bias=co_t[:, 3 * k + 2 : 3 * k + 3])
                xc.append(t0)
            y1_t = bp.tile([128, G], f32)
            for c in range(G // 512):
                sl = slice(c * 512, (c + 1) * 512)
                ps = pp.tile([128, 512], f32, tag="ps")
                nc.tensor.matmul(ps[:], w0_t[:], xc[0][:, sl], start=True, stop=False)
                nc.tensor.matmul(ps[:], w1_t[:], xc[1][:, sl], start=False, stop=True)
                nc.vector.tensor_scalar(out=y1_t[:, sl], in0=ps[:], scalar1=b_t[:, 0:1],
                                        scalar2=None, op0=mybir.AluOpType.add)
            sq_t = bp.tile([128, G], f32)
            nc.vector.tensor_tensor(out=sq_t[:], in0=y1_t[:], in1=y1_t[:],
                                    op=mybir.AluOpType.mult)
            st3_t = cp.tile([128, 2], f32)
            nc.vector.tensor_reduce(out=st3_t[:, 0:1], in_=y1_t[:],
                                    axis=mybir.AxisListType.X, op=mybir.AluOpType.add)
            nc.vector.tensor_reduce(out=st3_t[:, 1:2], in_=sq_t[:],
                                    axis=mybir.AxisListType.X, op=mybir.AluOpType.add)
            nc.sync.dma_start(st3[:], st3_t[:])
            nc.sync.dma_start(y1[:], y1_t[:])
    nc.compile()
    return nc


def _build_l4():
    """BN3+relu, lin2+relu, lin3."""
    bass, mybir, tile = _get_bass()
    f32 = mybir.dt.float32
    nc = bass.Bacc(None, target_bir_lowering=False)
    y1 = nc.dram_tensor("y1", [128, G], f32, kind="ExternalInput")
    g3b3 = nc.dram_tensor("g3b3", [128, 2], f32, kind="ExternalInput")
    w2 = nc.dram_tensor("w2", [128, 32], f32, kind="ExternalInput")
    l2b = nc.dram_tensor("l2b", [32, 1], f32, kind="ExternalInput")
    w3 = nc.dram_tensor("w3", [32, 2], f32, kind="ExternalInput")
    l3b = nc.dram_tensor("l3b", [2, 1], f32, kind="ExternalInput")
    yout = nc.dram_tensor("yout", [2, G], f32, kind="ExternalOutput")
    with tile.TileContext(nc) as tc:
        with (
            tc.tile_pool(name="const", bufs=1) as cp,
            tc.tile_pool(name="ps", bufs=4, space="PSUM") as pp,
            tc.tile_pool(name="big", bufs=1) as bp,
        ):
            y1_t = bp.tile([128, G], f32)
            nc.sync.dma_start(y1_t[:], y1[:])
            gb_t = cp.tile([128, 2], f32)
            nc.sync.dma_start(gb_t[:], g3b3[:])
            w2_t = cp.tile([128, 32], f32)
            nc.sync.dma_start(w2_t[:], w2[:])
            b2_t = cp.tile([32, 1], f32)
            nc.sync.dma_start(b2_t[:], l2b[:])
            w3_t = cp.tile([32, 2], f32)
            nc.sync.dma_start(w3_t[:], w3[:])
            b3_t = cp.tile([2, 1], f32)
            nc.sync.dma_start(b3_t[:], l3b[:])
            x2_t = bp.tile([128, G], f32)
            nc.vector.tensor_scalar(out=x2_t[:], in0=y1_t[:], scalar1=gb_t[:, 0:1],
                                    scalar2=None, op0=mybir.AluOpType.mult)
            nc.scalar.activation(x2_t[:], x2_t[:],
                                 mybir.ActivationFunctionType.Relu,
                                 bias=gb_t[:, 1:2])
            x3_t = bp.tile([32, G], f32)
            yo_t = bp.tile([2, G], f32)
            for c in range(G // 512):
                sl = slice(c * 512, (c + 1) * 512)
                ps2 = pp.tile([32, 512], f32, tag="ps2")
                nc.tensor.matmul(ps2[:], w2_t[:], x2_t[:, sl], start=True, stop=True)
                nc.scalar.activation(x3_t[:, sl], ps2[:],
                                     mybir.ActivationFunctionType.Relu,
                                     bias=b2_t[:, 0:1])
            for c in range(G // 512):
                sl = slice(c * 512, (c + 1) * 512)
                ps3 = pp.tile([2, 512], f32, tag="ps3")
                nc.tensor.matmul(ps3[:], w3_t[:], x3_t[:, sl], start=True, stop=True)
                nc.vector.tensor_scalar(out=yo_t[:, sl], in0=ps3[:],
                                        scalar1=b3_t[:, 0:1], scalar2=None,
                                        op0=mybir.AluOpType.add)
            nc.sync.dma_start(yout[:], yo_t[:])
    nc.compile()
    return nc


def _get_kernels():
    if "k" not in _KERNEL_CACHE:
        _KERNEL_CACHE["k"] = (_build_l1(NBLK), None,
                              _build_l2(), _build_l3(), _build_l4())
    return _KERNEL_CACHE["k"]


def _run(nc, in_maps, tag):
    from concourse.bass_utils import run_bass_kernel_spmd

    res = run_bass_kernel_spmd(nc, in_maps, core_ids=list(range(NCORES)))
    return res.results


# --------------------------------------------------------------------------
# main entry
# --------------------------------------------------------------------------
def kernel(**inputs) -> np.ndarray:
    x = np.asarray(inputs["x"], np.float32)
    eif = np.asarray(inputs["edge_index_func"])
    eis = np.asarray(inputs["edge_index_struct"])
    ew = np.asarray(inputs["edge_weight_func"], np.float32)
    W1 = np.asarray(inputs["W1"], np.float32)
    b1 = np.asarray(inputs["b1"], np.float32)
    g1 = np.asarray(inputs["g1"], np.float32)
    bt1 = np.asarray(inputs["bt1"], np.float32)
    W2 = np.asarray(inputs["W2"], np.float32)
    b2 = np.asarray(inputs["b2"], np.float32)
    g2 = np.asarray(inputs["g2"], np.float32)
    bt2 = np.asarray(inputs["bt2"], np.float32)
    lin1_W = np.asarray(inputs["lin1_W"], np.float32)
    lin1_b = np.asarray(inputs["lin1_b"], np.float32)
    g3 = np.asarray(inputs["g3"], np.float32)
    bt3 = np.asarray(inputs["bt3"], np.float32)
    lin2_W = np.asarray(inputs["lin2_W"], np.float32)
    lin2_b = np.asarray(inputs["lin2_b"], np.float32)
    lin3_W = np.asarray(inputs["lin3_W"], np.float32)
    lin3_b = np.asarray(inputs["lin3_b"], np.float32)

    sf, df = eif[0].astype(np.int64), eif[1].astype(np.int64)
    ss, ds = eis[0].astype(np.int64), eis[1].astype(np.int64)

    # --- structural-assumption checks (else exact numpy fallback) ---
    gs = ss // NN
    ok = np.array_equal(gs, ds // NN) and np.array_equal(
        gs, np.repeat(np.arange(B), ES)
    )
    gf = sf // NN
    ok = ok and np.array_equal(gf, df // NN) and np.array_equal(
        gf, np.repeat(np.arange(B), EF)
    )
    ssl, dsl = ss % NN, ds % NN
    ok = ok and np.array_equal(ssl.reshape(B, ES), np.broadcast_to(ssl[:ES], (B, ES)))
    ok = ok and np.array_equal(dsl.reshape(B, ES), np.broadcast_to(dsl[:ES], (B, ES)))
    ok = ok and np.abs(bt1).max() == 0.0
    if not ok:
        return _fallback_numpy(inputs)

    # --- host: build normalized func adjacency (transposed, self-loop folded)
    deg_f = np.bincount(df, weights=ew.astype(np.float64), minlength=N) + 1.0
    dinv_f = (1.0 / np.sqrt(deg_f)).astype(np.float32)
    norm_f = dinv_f[sf] * ew * dinv_f[df]
    sfl, dfl = sf % NN, df % NN
    idx = gf * (NN * NN) + sfl * NN + dfl
    AfT = np.bincount(idx, weights=norm_f.astype(np.float64),
                      minlength=B * NN * NN).astype(np.float32).reshape(B, NN, NN)
    dd = (dinv_f * dinv_f).reshape(B, NN)
    AfT[:, np.arange(NN), np.arange(NN)] += dd

    # --- host: shared structural adjacency (identical for all graphs)
    s0, d0 = ssl[:ES], dsl[:ES]
    deg_s = np.bincount(d0, minlength=NN).astype(np.float64) + 1.0
    dinv_s = 1.0 / np.sqrt(deg_s)
    AsT = np.zeros((NN, NN), np.float64)
    np.add.at(AsT, (s0, d0), dinv_s[s0] * dinv_s[d0])
    AsT[np.arange(NN), np.arange(NN)] += dinv_s * dinv_s
    asb = np.zeros((P114, 128), np.float32)
    for p in range(SLOT):
        asb[p * NN : (p + 1) * NN, p * NN : (p + 1) * NN] = AsT
    asb = asb.astype(BF16)

    # --- host: per-core packed inputs for L1
    x3 = x.reshape(B, NN, BANDS)
    l1_maps = []
    for c in range(NCORES):
        xs = np.zeros((NSLOT, NN, BANDS), np.float32)
        xs[:G] = x3[c * G : (c + 1) * G]
        xb = np.ascontiguousarray(
            xs.reshape(NBLK, SLOT, NN, BANDS).transpose(1, 2, 0, 3).reshape(
                P114, NBLK, BANDS)
        ).astype(BF16)
        Ac = np.zeros((NSLOT, NN, NN), np.float32)
        Ac[:G] = AfT[c * G : (c + 1) * G]
        Ac = Ac.reshape(NBLK, SLOT, NN, NN)
        Z = np.zeros((NBLK, P114, 128), np.float32)
        for p in range(SLOT):
            Z[:, p * NN : (p + 1) * NN, p * NN : (p + 1) * NN] = Ac[:, p]
        af = np.ascontiguousarray(Z.transpose(1, 0, 2)).astype(BF16)
        l1_maps.append({"xb": xb, "af": af})

    try:
        return _device_pipeline(l1_maps, asb, AsT.sum(0).astype(np.float64),
                                W1, g1, W2, b2, g2, bt2,
                                lin1_W, lin1_b, g3, bt3, lin2_W, lin2_b,
                                lin3_W, lin3_b)
    except Exception as e:
        import traceback
        print(f"device pipeline failed ({e}); numpy fallback", file=sys.stderr)
        traceback.print_exc()
        return _fallback_numpy(inputs)


def _device_pipeline(l1_maps, asb, cs, W1, g1, W2, b2, g2, bt2, lin1_W, lin1_b,
                     g3, bt3, lin2_W, lin2_b, lin3_W, lin3_b):
    ncs = _get_kernels()
    r1 = _run(ncs[0], l1_maps, "l1")

    # --- host: BN1 statistics + mu tile
    st = sum(r["st1"][:, 0].astype(np.float64) for r in r1)
    mu1 = (st[:BANDS] / N).astype(np.float32)
    var1 = (st[BANDS : 2 * BANDS] / N - mu1.astype(np.float64) ** 2).astype(np.float32)
    # h1 = s*W1row + b1 -> BN1 -> relu -> @W2 collapses to P*relu(z)+Q*relu(-z)
    w1r = W1[:, 0, :]                                # [BANDS, 32]
    rs1 = 1.0 / np.sqrt(var1[:, None] * w1r * w1r + EPS)   # [BANDS, 32]
    a = w1r * rs1 * g1                               # [BANDS, 32]
    Pk = np.einsum("bj,bjk->bk", np.maximum(a, 0), W2)     # [BANDS, 2]
    Qk = np.einsum("bj,bjk->bk", np.maximum(-a, 0), W2)    # [BANDS, 2]
    mub = np.broadcast_to(mu1[None, None, :], (128, 1, BANDS)).astype(np.float32)
    mub = np.ascontiguousarray(mub)

    l2_maps = [{"s_in": np.ascontiguousarray(r["s_out"].reshape(128, NBLK, BANDS)),
                "mub": mub, "asb": asb} for r in r1]
    r2 = _run(ncs[2], l2_maps, "l2")

    # --- host: BN2 statistics -> affine coefficients on (v+, v-)
    st2 = sum(r["st2"][:, 0].astype(np.float64) for r in r2)
    npad = NCORES * (NSLOT - G)            # pad graph slots across cores
    for b in range(BANDS):
        up_c = max(-float(mu1[b]), 0.0)
        um_c = max(float(mu1[b]), 0.0)
        svp, svm = up_c * cs, um_c * cs
        st2[0 + b] -= npad * svp.sum()
        st2[5 + b] -= npad * svm.sum()
        st2[10 + b] -= npad * (svp ** 2).sum()
        st2[15 + b] -= npad * (svm ** 2).sum()
        st2[20 + b] -= npad * (svp * svm).sum()
    mVp, mVm = st2[0:5] / N, st2[5:10] / N
    eVp2, eVm2, eVpm = st2[10:15] / N, st2[15:20] / N, st2[20:25] / N
    vVp = eVp2 - mVp**2
    vVm = eVm2 - mVm**2
    cVpm = eVpm - mVp * mVm
    # h2_k = Pk*v+ + Qk*v- + b2_k
    mu2 = Pk * mVp[:, None] + Qk * mVm[:, None] + b2          # [BANDS, 2]
    var2 = (Pk**2 * vVp[:, None] + Qk**2 * vVm[:, None]
            + 2 * Pk * Qk * cVpm[:, None])
    rs2 = 1.0 / np.sqrt(var2 + EPS)
    Ak = (Pk * rs2 * g2).astype(np.float32)                   # [BANDS, 2]
    Bk = (Qk * rs2 * g2).astype(np.float32)
    Ck = ((b2 - mu2) * rs2 * g2 + bt2).astype(np.float32)
    coef = np.zeros((95, 8), np.float32)
    for k in range(2):
        coef[:, 3 * k + 0] = np.repeat(Ak[:, k], NN)
        coef[:, 3 * k + 1] = np.repeat(Bk[:, k], NN)
        coef[:, 3 * k + 2] = np.repeat(Ck[:, k], NN)
    # lin1 row split by k-parity: row(band, n, k) = band*38 + n*2 + k
    ridx = (np.arange(BANDS)[:, None] * 2 * NN
            + np.arange(NN)[None, :] * 2).reshape(-1)         # [95]
    w1k0 = np.ascontiguousarray(lin1_W[ridx]).astype(np.float32)
    w1k1 = np.ascontiguousarray(lin1_W[ridx + 1]).astype(np.float32)
    l1bv = lin1_b.reshape(128, 1).astype(np.float32)

    l3_maps = []
    for c in range(NCORES):
        vo = r2[c]["v_out"].reshape(128, NBLK, 2, BANDS)[:P114]
        vo = vo.reshape(SLOT, NN, NBLK, 2, BANDS)
        # -> [band, n, pm, block, slot] -> graphs
        vp = vo.transpose(4, 1, 3, 2, 0).reshape(BANDS, NN, 2, NSLOT)[:, :, :, :G]
        v0p = np.ascontiguousarray(vp[:, :, 0, :].reshape(95, G))
        v1p = np.ascontiguousarray(vp[:, :, 1, :].reshape(95, G))
        l3_maps.append({"v0p": v0p, "v1p": v1p, "coef": coef,
                        "w1k0": w1k0, "w1k1": w1k1, "l1b": l1bv})
    r3 = _run(ncs[3], l3_maps, "l3")

    # --- host: BN3 statistics
    st3 = sum(r["st3"].astype(np.float64) for r in r3)
    mu3 = st3[:, 0] / B
    var3 = st3[:, 1] / B - mu3**2
    G3 = (g3 / np.sqrt(var3 + EPS)).astype(np.float32)
    B3 = (bt3 - mu3 * G3).astype(np.float32)
    g3b3 = np.ascontiguousarray(np.stack([G3, B3], axis=1))
    l4_maps = [{"y1": r["y1"], "g3b3": g3b3,
                "w2": np.ascontiguousarray(lin2_W),
                "l2b": lin2_b.reshape(32, 1),
                "w3": np.ascontiguousarray(lin3_W),
                "l3b": lin3_b.reshape(2, 1)} for r in r3]
    r4 = _run(ncs[4], l4_maps, "l4")

    out = np.empty((B, 2), np.float32)
    for c in range(NCORES):
        out[c * G : (c + 1) * G] = r4[c]["yout"].T
    return out

